# revision 25
# baseline (speedup 1.0000x reference)
"""Trainium2 Bass kernel: 4-head transformer core (attention + residual + LayerNorm).

Reference computation (per batch b of 4, seq 2048, d_model 1024, 4 heads x 256):
    qkv = x @ qkv_w.T + qkv_b ; q,k,v per head
    attn = softmax(q k^T / 16) ; out = attn v
    y = x + out @ wo_w.T + wo_b ; layernorm(y) * gamma + beta

Sharding: pure data parallel over (batch, seq-half) -> 8 cores, no collectives.
Each core handles 1024 query tokens of one batch; K/V are computed for the
full 2048 tokens of that batch (duplicated across the 2 cores of a batch).
Host passes x pre-transposed (d-major) and rotated so the core's local tokens
are always columns [0, 1024) -- one SPMD program serves all cores.  Attention
is permutation-invariant over key/value positions, so the rotation does not
change the result.

All matmuls run in fp8e4 (e4m3) with DoubleRow perf mode: each instruction
contracts K=256 (two 128-partition subtiles packed in the free dim of both
operands) at the same per-instruction cost as one bf16 K=128 matmul -- 2x
the effective PE throughput.  The attention output is diluted ~25x by the
f32 residual before LayerNorm, so fp8 quantization of x/q/k/v/es/wo keeps
the final relative error ~4e-3, well inside the 2e-2 gate.  exp carries a
-ln(32) bias so its fp8 output cannot overflow (hw fp8e4 infs above ~240);
the softmax normalization cancels the factor exactly.

Schedule: ONE fully interleaved instruction stream.  Attention units run
qc-outer / h-inner; the QKV projection chunks for head h+1 (and the V
chunks) are paced INSIDE unit h's j-loop so the PE never waits on the ACT
exp chain, and the wo projection + LayerNorm for the qc0 token tiles are
paced inside the qc1 units.  Only the last 4 token tiles drain after the
final attention unit.  PSUM: scores 2 banks, av accumulators 4, misc
(QKV/V/wo chunks + softmax denominator) 2.

Softmax denominator: DoubleRow ones-matmul broadcasts den to all 128
partitions; 1/den via the fast DVE Newton reciprocal; out^T = av * (1/den)
on DVE (fp8 out).  LayerNorm rstd = exp(-0.5*ln(var+eps)) on ACT -- Ln and
Exp live in the same ACT table set, so no table reloads mid-kernel.
"""

import os

import ml_dtypes
import numpy as np

P = 128
B, S, D = 4, 2048, 1024
H = 4
HD = D // H  # 256
SL = S // 2  # local query tokens per core
DC = D // P  # 8 d-chunks
QT_TILES = SL // P  # 8
KT_TILES = S // P  # 16
NQ = SL // 512  # 2 q-chunks of 512
EPS = 1e-5
NCORES = 8

_BF16 = ml_dtypes.bfloat16
_FP8 = ml_dtypes.float8_e4m3fn

_CACHE = {}


def _install_drain_patch():
    """walrus CoreV3 in this container accepts at most one sem wait per SP
    CTRL instruction, but Tile's kernel-tail drain carries one wait per
    outstanding logical proc.  Redistribute them onto single-wait no-ops."""
    import concourse.tile as _tile
    from concourse import mybir
    from concourse.vector_clock import ScopedClock

    if getattr(_tile.TileContext, "_drain_patch_installed", False):
        return

    def _drain_and_barrier(self, tick_clock, wait_clock):
        nc = self.nc
        drain_inst = nc.sync.drain()
        wait_clock.add_sem_waits(
            drain_inst.ins, ScopedClock({None: tick_clock.global_clock})
        )
        si = drain_inst.ins.sync_info
        if si is not None and len(si.on_wait) > 1:
            waits = list(si.on_wait)
            drain_inst.ins.sync_info = mybir.SyncInfo(
                on_wait=[], on_update=list(si.on_update)
            )
            for w in waits:
                nop = nc.sync.nop(nofuse=True, hint="drain_wait_split")
                nop.ins.sync_info = mybir.SyncInfo(on_wait=[w], on_update=[])

        nc.all_engine_barrier()
        assert self.sems is not None
        popped = nc._tile_sem_poison_stack.pop()
        assert popped is self._sem_poison
        nc.clear_and_free_semaphores(list(self.sems.allocated().values()))
        nc.all_engine_barrier()

    _tile.TileContext._drain_and_barrier = _drain_and_barrier
    _tile.TileContext._drain_patch_installed = True


def _split_excess_waits(nc):
    """This walrus build accepts at most one sem wait per instruction (two for
    EventSemaphore), but Tile attaches one wait per outstanding proc.  Move
    the excess waits onto same-engine no-ops inserted just before each
    over-subscribed instruction (same-engine program order makes the waits
    complete before the instruction issues)."""
    from concourse import mybir

    n_split = 0
    for f in nc.m.functions:
        for b in f.blocks:
            insts = b.instructions
            new_list = []
            changed = False
            for inst in insts:
                si = inst.sync_info
                cap = 2 if isinstance(inst, mybir.InstEventSemaphore) else 1
                if si is not None and len(si.on_wait) > cap:
                    waits = list(si.on_wait)
                    for k, w in enumerate(waits[:-cap]):
                        nop = mybir.InstNoOp(name=f"{inst.name}-ws{k}")
                        nop.engine = inst.engine
                        nop.bass_nofuse = True
                        nop.sync_info = mybir.SyncInfo(on_wait=[w], on_update=[])
                        new_list.append(nop)
                        n_split += 1
                    inst.sync_info = mybir.SyncInfo(
                        on_wait=waits[-cap:], on_update=list(si.on_update)
                    )
                    changed = True
                new_list.append(inst)
            if changed:
                b.instructions = new_list
    return n_split


def _build(ZB=False):
    """ZB: specialize for qkv_b == 0, wo_b folded on host, gamma == 1, beta == 0."""
    import concourse.bass as bass
    import concourse.tile as tile
    from concourse import mybir

    _install_drain_patch()

    f32 = mybir.dt.float32
    fp8 = mybir.dt.float8e4
    AF = mybir.ActivationFunctionType
    ALU = mybir.AluOpType
    DR = mybir.MatmulPerfMode.DoubleRow

    nc = bass.Bass()

    # Host pre-swizzles every input so each DMA descriptor covers a 1-2KB
    # contiguous run (the DMA engines are descriptor-rate-bound, not
    # byte-bound, for the fine-grained layouts).
    xT_d = nc.dram_tensor("xT", [P, DC, S], fp8, kind="ExternalInput")
    wqkT_d = nc.dram_tensor("wqkT", [P, 16, DC, P], fp8, kind="ExternalInput")
    wv_d = nc.dram_tensor("wv", [P, DC, D], fp8, kind="ExternalInput")
    woT_d = nc.dram_tensor("woT", [D, D], fp8, kind="ExternalInput")
    xb_d = nc.dram_tensor("xb", [SL, D], f32, kind="ExternalInput")
    qkvb_d = nc.dram_tensor("qkvb", [P, 24], f32, kind="ExternalInput")
    vb_d = nc.dram_tensor("vb", [P, D], f32, kind="ExternalInput")
    gamma_d = nc.dram_tensor("gamma", [P, D], f32, kind="ExternalInput")
    beta_d = nc.dram_tensor("beta", [P, D], f32, kind="ExternalInput")
    y_d = nc.dram_tensor("y", [SL, D], f32, kind="ExternalOutput")

    with (
        tile.TileContext(nc) as tc,
        nc.allow_low_precision(reason="fp8 attention path, tolerance 2e-2"),
        tc.tile_pool(name="persist", bufs=1) as pp,
        tc.tile_pool(name="es_pool", bufs=2) as pes,
        tc.tile_pool(name="bc_pool", bufs=2) as pbc,
        tc.tile_pool(name="y_pool", bufs=3) as pdy,
        tc.tile_pool(name="st_pool", bufs=4) as pst,
        tc.tile_pool(name="ps_sc", bufs=2, space="PSUM") as ps_sc,
        tc.tile_pool(name="ps_out", bufs=4, space="PSUM") as ps_out,
        tc.tile_pool(name="ps_misc", bufs=2, space="PSUM") as ps_misc,
    ):
        qT = pp.tile([P, DC, SL], fp8, tag="qT")
        kT = pp.tile([P, DC, S], fp8, tag="kT")
        v = pp.tile([P, KT_TILES, D], fp8, tag="v")
        outT = pp.tile([P, DC, SL], fp8, tag="outT")
        woT = pp.tile([P, DC, D], fp8, tag="woT")
        xT = pp.tile([P, DC, S], fp8, tag="xT")
        wv = pp.tile([P, DC, D], fp8, tag="wv")
        wqk = [pp.tile([P, DC, P], fp8, name=f"wqk{m}", tag=f"wqk{m}") for m in range(16)]
        if not ZB:
            gamma_bc = pp.tile([P, D], f32, tag="gamma_bc")
            beta_bc = pp.tile([P, D], f32, tag="beta_bc")
            vb_bc = pp.tile([P, D], f32, tag="vb_bc")
            qkvb = pp.tile([P, 24], f32, tag="qkvb")
        ones_k2 = pp.tile([P, 2, P], fp8, tag="ones_k2")
        eps_t = pp.tile([P, 1], f32, tag="eps")
        nln32 = pp.tile([P, 1], f32, tag="nln32")

        nc.vector.memset(ones_k2, 1.0)
        nc.vector.memset(eps_t, EPS)
        nc.vector.memset(nln32, -3.4657359027997265)

        # ---- input DMAs.  All 16 hw DMA engines are SHARED round-robin
        # across everything in flight, so the only way to get the critical
        # bytes early is to not issue the rest yet: wave 1 is exactly what
        # the first six projection chunks touch; everything else is issued
        # from the gpsimd queue behind a data-dependency gate on an early
        # chunk, so it only enters the fabric once wave 1 has landed.
        def xt_piece(eng, dcp, lo, hi):
            eng.dma_start(
                out=xT[:, 2 * dcp : 2 * dcp + 2, lo:hi],
                in_=xT_d[:, 2 * dcp : 2 * dcp + 2, lo:hi],
            )

        for m in (8, 9, 0, 1):
            nc.sync.dma_start(out=wqk[m], in_=wqkT_d[:, m])
        for dcp, eng in ((0, nc.scalar), (1, nc.scalar), (2, nc.gpsimd), (3, nc.gpsimd)):
            xt_piece(eng, dcp, 0, SL)
            xt_piece(eng, dcp, SL, S)
        dma_gate_scr = pp.tile([1, 1], f32, tag="dma_gate_scr")
        if not ZB:
            nc.gpsimd.dma_start(out=qkvb, in_=qkvb_d[:])
            nc.gpsimd.dma_start(out=vb_bc, in_=vb_d[:])
            nc.sync.dma_start(out=gamma_bc, in_=gamma_d[:])
            nc.sync.dma_start(out=beta_bc, in_=beta_d[:])

        # ---- chunk emitters (each: 4 DoubleRow matmuls + one PSUM->SBUF cast) ----
        def qkv_chunk(m, qc, eng, pool=None):
            """Q (m<8, 512 local tokens) or K (m>=8, 512 of 2048 tokens)."""
            pool = pool or ps_misc
            ps = pool.tile([P, 512], f32, name="ps_qk",
                           tag="ps_out" if pool is ps_out else "ps_misc")
            for dcp in range(DC // 2):
                nc.tensor.matmul(
                    ps,
                    lhsT=wqk[m][:, 2 * dcp : 2 * dcp + 2, :],
                    rhs=xT[:, 2 * dcp : 2 * dcp + 2, qc * 512 : (qc + 1) * 512],
                    start=(dcp == 0),
                    stop=(dcp == DC // 2 - 1),
                    perf_mode=DR,
                )
            if m < 8:
                dst = qT[:, m, qc * 512 : (qc + 1) * 512]
            else:
                dst = kT[:, m - 8, qc * 512 : (qc + 1) * 512]
            if ZB:
                if eng is nc.scalar:
                    nc.scalar.activation(
                        out=dst, in_=ps, func=AF.Identity, bias=0.0, scale=1.0
                    )
                else:
                    eng.tensor_copy(out=dst, in_=ps)
            else:
                nc.scalar.activation(
                    out=dst, in_=ps, func=AF.Identity, bias=qkvb[:, m : m + 1], scale=1.0
                )

        def v_chunk(vt, oc, eng, pool=None):
            pool = pool or ps_misc
            ps = pool.tile([P, 512], f32, name="ps_v",
                           tag="ps_out" if pool is ps_out else "ps_misc")
            for dcp in range(DC // 2):
                nc.tensor.matmul(
                    ps,
                    lhsT=xT[:, 2 * dcp : 2 * dcp + 2, vt * P : (vt + 1) * P],
                    rhs=wv[:, 2 * dcp : 2 * dcp + 2, oc * 512 : (oc + 1) * 512],
                    start=(dcp == 0),
                    stop=(dcp == DC // 2 - 1),
                    perf_mode=DR,
                )
            dst = v[:, vt, oc * 512 : (oc + 1) * 512]
            if ZB:
                if eng is nc.scalar:
                    nc.scalar.activation(
                        out=dst, in_=ps, func=AF.Identity, bias=0.0, scale=1.0
                    )
                else:
                    eng.tensor_copy(out=dst, in_=ps)
            else:
                nc.vector.tensor_add(
                    out=dst, in0=ps, in1=vb_bc[:, oc * 512 : (oc + 1) * 512]
                )

        y_tiles = {}

        def wo_chunk(qt, oc, eng, pool=None):
            if qt not in y_tiles:
                y_t = pdy.tile([P, D], f32, name=f"y{qt}", tag="y")
                y_tiles[qt] = y_t
                # residual base: y_t starts as x + wo_b (host-folded); the
                # wo partial sums are added from PSUM by the DVE below, so
                # nothing waits on a DMA in the LayerNorm critical chain.
                nc.gpsimd.dma_start(out=y_t, in_=xb_d[qt * P : (qt + 1) * P, :])
            y_t = y_tiles[qt]
            pool = pool or ps_misc
            ps = pool.tile([P, 512], f32, name="ps_wo",
                           tag="ps_out" if pool is ps_out else "ps_misc")
            for dcp in range(DC // 2):
                nc.tensor.matmul(
                    ps,
                    lhsT=outT[:, 2 * dcp : 2 * dcp + 2, qt * P : (qt + 1) * P],
                    rhs=woT[:, 2 * dcp : 2 * dcp + 2, oc * 512 : (oc + 1) * 512],
                    start=(dcp == 0),
                    stop=(dcp == DC // 2 - 1),
                    perf_mode=DR,
                )
            dst = y_t[:, oc * 512 : (oc + 1) * 512]
            nc.vector.tensor_add(out=dst, in0=ps, in1=dst)

        def ln_tile(qt):
            """LayerNorm + store for token tile qt (residual already in y_t)."""
            y_t = y_tiles.pop(qt)
            stats = pst.tile([P, 2, 6], f32, tag="stats")
            for sg in range(2):
                nc.vector.bn_stats(
                    out=stats[:, sg, :], in_=y_t[:, sg * 512 : (sg + 1) * 512]
                )
            mv = pst.tile([P, 2], f32, tag="mv")
            nc.vector.bn_aggr(out=mv, in_=stats)
            # rstd = exp(-0.5*ln(var+eps)): Ln and Exp share the ACT exp
            # table set, so this never forces a mid-kernel table reload.
            lnv = pst.tile([P, 1], f32, tag="lnv")
            nc.scalar.activation(
                out=lnv, in_=mv[:, 1:2], func=AF.Ln, bias=eps_t, scale=1.0
            )
            rstd = pst.tile([P, 1], f32, tag="rstd")
            nc.scalar.activation(out=rstd, in_=lnv, func=AF.Exp, bias=0.0, scale=-0.5)
            nc.vector.tensor_scalar(
                out=y_t,
                in0=y_t,
                scalar1=mv[:, 0:1],
                scalar2=rstd,
                op0=ALU.subtract,
                op1=ALU.mult,
            )
            if not ZB:
                nc.vector.tensor_mul(out=y_t, in0=y_t, in1=gamma_bc)
                nc.vector.tensor_add(out=y_t, in0=y_t, in1=beta_bc)
            # alternate store queues so the drain's four stores overlap
            st_eng = nc.sync if qt % 2 == 0 else nc.scalar
            st_eng.dma_start(out=y_d[qt * P : (qt + 1) * P, :], in_=y_t)

        # ---- softmax finalize (pipelined into the following unit) ----
        def finalize_den(p, on_act=True):
            es_p, ops_p, hqc = p
            dps = ps_misc.tile([P, 512], f32, name="dps", tag="ps_misc")
            for jp in range(KT_TILES // 2):
                nc.tensor.matmul(
                    dps,
                    lhsT=ones_k2,
                    rhs=es_p[:, 2 * jp : 2 * jp + 2, :],
                    start=(jp == 0),
                    stop=(jp == KT_TILES // 2 - 1),
                    perf_mode=DR,
                )
            # 1/den on whichever engine has slack in the surrounding unit:
            # exp(-ln(den)) on ACT (both funcs in the exp table set, no
            # reload) while the DVE is saturated with projection casts, or
            # the DVE reciprocal once ACT becomes the bottleneck.  LUT error
            # ~1e-3, diluted by the residual far below the 2e-2 gate.
            bc = pbc.tile([P, 512], f32, tag="bc")
            if on_act:
                lden = pbc.tile([P, 512], f32, tag="lden")
                nc.scalar.activation(out=lden, in_=dps, func=AF.Ln, bias=0.0, scale=1.0)
                nc.scalar.activation(out=bc, in_=lden, func=AF.Exp, bias=0.0, scale=-1.0)
            else:
                nc.vector.reciprocal(out=bc, in_=dps)
            return (ops_p, hqc, bc)

        def finalize_scale(p2):
            ops_p, (qc_p, h_p), bc = p2
            qsl_p = slice(qc_p * 512, (qc_p + 1) * 512)
            for f in range(2):
                nc.vector.tensor_mul(
                    out=outT[:, 2 * h_p + f, qsl_p], in0=ops_p[f], in1=bc
                )

        # ---- the interleaved main loop ----
        state = {"pend": None, "pend2": None}

        def unit(qc, h, carried, start_at=0, bc_on_act=True):
            """One attention unit (512 q tokens x head h), with `carried`
            thunks (projection/wo/LN work) paced across iterations
            [start_at, 16) of its j-loop."""
            n_car = len(carried)
            emitted = 0
            qsl = slice(qc * 512, (qc + 1) * 512)
            es = pes.tile([P, KT_TILES, 512], fp8, tag="es")
            op0 = ps_out.tile([P, 512], f32, tag="ps_out")
            op1 = ps_out.tile([P, 512], f32, tag="ps_out")
            ops = [op0, op1]
            for j in range(KT_TILES):
                sps = ps_sc.tile([P, 512], f32, tag="ps_sc")
                nc.tensor.matmul(
                    sps,
                    lhsT=kT[:, 2 * h : 2 * h + 2, j * P : (j + 1) * P],
                    rhs=qT[:, 2 * h : 2 * h + 2, qsl],
                    start=True,
                    stop=True,
                    perf_mode=DR,
                )
                nc.scalar.activation(
                    out=es[:, j, :],
                    in_=sps,
                    func=AF.Exp,
                    bias=nln32,
                    scale=1.0 / 16.0,
                )
                if j == 4 and state["pend"] is not None:
                    state["pend2"] = finalize_den(state["pend"], on_act=bc_on_act)
                    state["pend"] = None
                if j == 11 and state["pend2"] is not None:
                    finalize_scale(state["pend2"])
                    state["pend2"] = None
                want = n_car * max(0, j + 1 - start_at) // (KT_TILES - start_at)
                while emitted < want:
                    carried[emitted]()
                    emitted += 1
                if j % 2 == 1:
                    jp = j // 2
                    for f in range(2):
                        nc.tensor.matmul(
                            ops[f],
                            lhsT=v[
                                :,
                                2 * jp : 2 * jp + 2,
                                h * HD + f * P : h * HD + (f + 1) * P,
                            ],
                            rhs=es[:, 2 * jp : 2 * jp + 2, :],
                            start=(jp == 0),
                            stop=(jp == KT_TILES // 2 - 1),
                            perf_mode=DR,
                        )
            while emitted < n_car:
                carried[emitted]()
                emitted += 1
            state["pend"] = (es, ops, (qc, h))

        A, V_ = nc.scalar, nc.vector

        # PRE: everything unit (qc0,h0) needs up front.
        pre = [
            lambda p: qkv_chunk(8, 0, A, p),
            lambda p: qkv_chunk(9, 0, V_, p),
            lambda p: qkv_chunk(0, 0, A, p),
            lambda p: qkv_chunk(1, 0, V_, p),
            lambda p: qkv_chunk(8, 1, A, p),
            lambda p: qkv_chunk(9, 1, V_, p),
            lambda p: qkv_chunk(8, 2, A, p),
            lambda p: qkv_chunk(9, 2, V_, p),
            lambda p: qkv_chunk(8, 3, A, p),
            lambda p: qkv_chunk(9, 3, V_, p),
            lambda p: v_chunk(0, 0, A, p),
            lambda p: v_chunk(1, 0, V_, p),
            lambda p: v_chunk(2, 0, A, p),
            lambda p: v_chunk(3, 0, V_, p),
        ]
        for i, t in enumerate(pre):
            t(ps_out if i % 3 == 2 else ps_misc)
            if i == 1:
                # gate: gpsimd stalls here until this PRE tile exists, so the
                # wave-2 DMAs below don't compete with wave 1 for the fabric.
                nc.gpsimd.tensor_copy(out=dma_gate_scr, in_=kT[0:1, 1, 0:1])
                nc.gpsimd.dma_start(out=wv, in_=wv_d[:])
                for m in (10, 11, 2, 3):
                    nc.gpsimd.dma_start(out=wqk[m], in_=wqkT_d[:, m])

        def carry_k(h2):  # K chunks for head h2 (m = 8+2*h2, 9+2*h2), token order
            out = []
            for qc2 in range(4):
                out.append(lambda m=8 + 2 * h2, q2=qc2: qkv_chunk(m, q2, V_))
                out.append(lambda m=9 + 2 * h2, q2=qc2: qkv_chunk(m, q2, V_))
            return out

        def carry_q(h2, qc2):
            return [
                lambda m=2 * h2: qkv_chunk(m, qc2, V_),
                lambda m=2 * h2 + 1: qkv_chunk(m, qc2, V_),
            ]

        def carry_v(vts, oc):
            return [lambda t=t_, o=oc: v_chunk(t, o, V_) for t_ in vts]

        carried_by_unit = [
            # (qc0,h0): rest of v oc0 (self, paced ahead of av) + head1 k/q
            carry_v(range(4, 16), 0) + carry_k(1) + carry_q(1, 0),
            # (qc0,h1): v oc1 first half + head2 k/q
            carry_v(range(0, 8), 1) + carry_k(2) + carry_q(2, 0),
            # (qc0,h2): v oc1 second half (self-paced, needed from jp>=4) + head3 k/q
            carry_v(range(8, 16), 1) + carry_k(3) + carry_q(3, 0),
            # (qc0,h3): all qc1 q chunks
            [t for h2 in range(4) for t in carry_q(h2, 1)],
            # (qc1,h0): nothing until the qc0 finalize lands (j==11); wo qt0
            [lambda: wo_chunk(0, 0, V_), lambda: wo_chunk(0, 1, V_), lambda: ln_tile(0)],
            # (qc1,h1): wo qt1
            [lambda: wo_chunk(1, 0, V_), lambda: wo_chunk(1, 1, V_), lambda: ln_tile(1)],
            # (qc1,h2): wo qt2
            [lambda: wo_chunk(2, 0, V_), lambda: wo_chunk(2, 1, V_), lambda: ln_tile(2)],
            # (qc1,h3): wo qt3
            [lambda: wo_chunk(3, 0, V_), lambda: wo_chunk(3, 1, V_), lambda: ln_tile(3)],
        ]
        units = [(qc, h) for qc in range(NQ) for h in range(H)]
        for i, ((qc, h), carried) in enumerate(zip(units, carried_by_unit, strict=True)):
            if i == 1:
                for m in (12, 13, 4, 5, 14, 15, 6, 7):
                    nc.gpsimd.dma_start(out=wqk[m], in_=wqkT_d[:, m])
                nc.gpsimd.dma_start(
                    out=woT, in_=woT_d[:].rearrange("(c p) o -> p c o", p=P)
                )
            # (qc1,h0)'s wo work is legal only after the (qc0,h3)
            # finalize_scale, which this unit emits at j==11.
            unit(
                qc,
                h,
                carried,
                start_at=12 if (qc, h) == (1, 0) else 0,
                bc_on_act=(i < 4),
            )

        # drain: finalize last unit, then the remaining 4 token tiles.
        finalize_scale(finalize_den(state["pend"], on_act=True))
        for qt in range(4, 8):
            wo_chunk(qt, 0, A, ps_out)
            wo_chunk(qt, 1, V_, ps_misc)
            ln_tile(qt)

    _split_excess_waits(nc)
    return nc


def _get_nc(ZB):
    key = ("nc", ZB)
    if key not in _CACHE:
        _CACHE[key] = _build(ZB)
    return _CACHE[key]


def kernel(x, qkv_w, qkv_b, wo_w, wo_b, gamma, beta):
    from concourse.bass_utils import run_bass_kernel_spmd

    x = np.asarray(x, dtype=np.float32)
    qkv_w = np.asarray(qkv_w, dtype=np.float32)
    qkv_b = np.asarray(qkv_b, dtype=np.float32)
    wo_w = np.asarray(wo_w, dtype=np.float32)
    wo_b = np.asarray(wo_b, dtype=np.float32)
    gamma = np.asarray(gamma, dtype=np.float32)
    beta = np.asarray(beta, dtype=np.float32)

    W3 = qkv_w.T.reshape(DC, P, 3 * D)  # [c, p, f]
    wqkT = np.ascontiguousarray(
        W3[:, :, : 2 * D].reshape(DC, P, 16, P).transpose(1, 2, 0, 3)
    ).astype(_FP8)  # [p, m, c, j]
    wv_h = np.ascontiguousarray(W3[:, :, 2 * D :].transpose(1, 0, 2)).astype(_FP8)
    woT = np.ascontiguousarray(wo_w.T).astype(_FP8)
    qkvb2d = np.ascontiguousarray(qkv_b.reshape(24, P).T)
    vb = np.ascontiguousarray(np.broadcast_to(qkv_b[2 * D :], (P, D))).astype(np.float32)
    gamma_r = np.ascontiguousarray(np.broadcast_to(gamma, (P, D))).astype(np.float32)
    beta_r = np.ascontiguousarray(np.broadcast_to(beta, (P, D))).astype(np.float32)

    in_maps = []
    for c in range(NCORES):
        b, t = divmod(c, 2)
        loc = x[b, t * SL : (t + 1) * SL]  # [SL, D]
        oth = x[b, (1 - t) * SL : (2 - t) * SL]
        xT = np.concatenate([loc.T, oth.T], axis=1)  # [D, S]
        xT = np.ascontiguousarray(xT.reshape(DC, P, S).transpose(1, 0, 2)).astype(_FP8)
        xb = (loc + wo_b[None, :]).astype(np.float32)
        in_maps.append(
            {
                "xT": xT,
                "wqkT": wqkT,
                "wv": wv_h,
                "woT": woT,
                "xb": xb,
                "qkvb": qkvb2d,
                "vb": vb,
                "gamma": gamma_r,
                "beta": beta_r,
            }
        )

    zb = (
        not qkv_b.any()
        and bool(np.all(gamma == 1.0))
        and not beta.any()
    )
    trace = os.environ.get("BASS_KERNEL_TRACE") == "1"
    res = run_bass_kernel_spmd(
        _get_nc(zb),
        in_maps,
        list(range(NCORES)),
        trace=trace,
        tmpdir=os.environ.get("BASS_KERNEL_TRACE_DIR") or None,
    )
    _CACHE["last_result"] = res

    out = np.empty((B, S, D), dtype=np.float32)
    for c in range(NCORES):
        b, t = divmod(c, 2)
        out[b, t * SL : (t + 1) * SL] = res.results[c]["y"]
    return out


# revision 26
# speedup vs baseline: 1.0084x; 1.0084x over previous
"""Trainium2 Bass kernel: 4-head transformer core (attention + residual + LayerNorm).

Reference computation (per batch b of 4, seq 2048, d_model 1024, 4 heads x 256):
    qkv = x @ qkv_w.T + qkv_b ; q,k,v per head
    attn = softmax(q k^T / 16) ; out = attn v
    y = x + out @ wo_w.T + wo_b ; layernorm(y) * gamma + beta

Sharding: pure data parallel over (batch, seq-half) -> 8 cores, no collectives.
Each core handles 1024 query tokens of one batch; K/V are computed for the
full 2048 tokens of that batch (duplicated across the 2 cores of a batch).
Host passes x pre-transposed (d-major) and rotated so the core's local tokens
are always columns [0, 1024) -- one SPMD program serves all cores.  Attention
is permutation-invariant over key/value positions, so the rotation does not
change the result.

All matmuls run in fp8e4 (e4m3) with DoubleRow perf mode: each instruction
contracts K=256 (two 128-partition subtiles packed in the free dim of both
operands) at the same per-instruction cost as one bf16 K=128 matmul -- 2x
the effective PE throughput.  The attention output is diluted ~25x by the
f32 residual before LayerNorm, so fp8 quantization of x/q/k/v/es/wo keeps
the final relative error ~4e-3, well inside the 2e-2 gate.  exp carries a
-ln(32) bias so its fp8 output cannot overflow (hw fp8e4 infs above ~240);
the softmax normalization cancels the factor exactly.

Schedule: ONE fully interleaved instruction stream.  Attention units run
qc-outer / h-inner; the QKV projection chunks for head h+1 (and the V
chunks) are paced INSIDE unit h's j-loop so the PE never waits on the ACT
exp chain, and the wo projection + LayerNorm for the qc0 token tiles are
paced inside the qc1 units.  Only the last 4 token tiles drain after the
final attention unit.  PSUM: scores 2 banks, av accumulators 4, misc
(QKV/V/wo chunks + softmax denominator) 2.

Softmax denominator: DoubleRow ones-matmul broadcasts den to all 128
partitions; 1/den via the fast DVE Newton reciprocal; out^T = av * (1/den)
on DVE (fp8 out).  LayerNorm rstd = exp(-0.5*ln(var+eps)) on ACT -- Ln and
Exp live in the same ACT table set, so no table reloads mid-kernel.
"""

import os

import ml_dtypes
import numpy as np

P = 128
B, S, D = 4, 2048, 1024
H = 4
HD = D // H  # 256
SL = S // 2  # local query tokens per core
DC = D // P  # 8 d-chunks
QT_TILES = SL // P  # 8
KT_TILES = S // P  # 16
NQ = SL // 512  # 2 q-chunks of 512
EPS = 1e-5
NCORES = 8

_BF16 = ml_dtypes.bfloat16
_FP8 = ml_dtypes.float8_e4m3fn

_CACHE = {}


def _install_drain_patch():
    """walrus CoreV3 in this container accepts at most one sem wait per SP
    CTRL instruction, but Tile's kernel-tail drain carries one wait per
    outstanding logical proc.  Redistribute them onto single-wait no-ops."""
    import concourse.tile as _tile
    from concourse import mybir
    from concourse.vector_clock import ScopedClock

    if getattr(_tile.TileContext, "_drain_patch_installed", False):
        return

    def _drain_and_barrier(self, tick_clock, wait_clock):
        nc = self.nc
        drain_inst = nc.sync.drain()
        wait_clock.add_sem_waits(
            drain_inst.ins, ScopedClock({None: tick_clock.global_clock})
        )
        si = drain_inst.ins.sync_info
        if si is not None and len(si.on_wait) > 1:
            waits = list(si.on_wait)
            drain_inst.ins.sync_info = mybir.SyncInfo(
                on_wait=[], on_update=list(si.on_update)
            )
            for w in waits:
                nop = nc.sync.nop(nofuse=True, hint="drain_wait_split")
                nop.ins.sync_info = mybir.SyncInfo(on_wait=[w], on_update=[])

        nc.all_engine_barrier()
        assert self.sems is not None
        popped = nc._tile_sem_poison_stack.pop()
        assert popped is self._sem_poison
        nc.clear_and_free_semaphores(list(self.sems.allocated().values()))
        nc.all_engine_barrier()

    _tile.TileContext._drain_and_barrier = _drain_and_barrier
    _tile.TileContext._drain_patch_installed = True


def _split_excess_waits(nc):
    """This walrus build accepts at most one sem wait per instruction (two for
    EventSemaphore), but Tile attaches one wait per outstanding proc.  Move
    the excess waits onto same-engine no-ops inserted just before each
    over-subscribed instruction (same-engine program order makes the waits
    complete before the instruction issues)."""
    from concourse import mybir

    n_split = 0
    for f in nc.m.functions:
        for b in f.blocks:
            insts = b.instructions
            new_list = []
            changed = False
            for inst in insts:
                si = inst.sync_info
                cap = 2 if isinstance(inst, mybir.InstEventSemaphore) else 1
                if si is not None and len(si.on_wait) > cap:
                    waits = list(si.on_wait)
                    for k, w in enumerate(waits[:-cap]):
                        nop = mybir.InstNoOp(name=f"{inst.name}-ws{k}")
                        nop.engine = inst.engine
                        nop.bass_nofuse = True
                        nop.sync_info = mybir.SyncInfo(on_wait=[w], on_update=[])
                        new_list.append(nop)
                        n_split += 1
                    inst.sync_info = mybir.SyncInfo(
                        on_wait=waits[-cap:], on_update=list(si.on_update)
                    )
                    changed = True
                new_list.append(inst)
            if changed:
                b.instructions = new_list
    return n_split


def _build(ZB=False):
    """ZB: specialize for qkv_b == 0, wo_b folded on host, gamma == 1, beta == 0."""
    import concourse.bass as bass
    import concourse.tile as tile
    from concourse import mybir

    _install_drain_patch()

    f32 = mybir.dt.float32
    fp8 = mybir.dt.float8e4
    AF = mybir.ActivationFunctionType
    ALU = mybir.AluOpType
    DR = mybir.MatmulPerfMode.DoubleRow

    nc = bass.Bass()

    # Host pre-swizzles every input so each DMA descriptor covers a 1-2KB
    # contiguous run (the DMA engines are descriptor-rate-bound, not
    # byte-bound, for the fine-grained layouts).
    xT_d = nc.dram_tensor("xT", [P, DC, S], fp8, kind="ExternalInput")
    wqkT_d = nc.dram_tensor("wqkT", [P, 16, DC, P], fp8, kind="ExternalInput")
    wv_d = nc.dram_tensor("wv", [P, DC, D], fp8, kind="ExternalInput")
    woT_d = nc.dram_tensor("woT", [D, D], fp8, kind="ExternalInput")
    xb_d = nc.dram_tensor("xb", [SL, D], f32, kind="ExternalInput")
    qkvb_d = nc.dram_tensor("qkvb", [P, 24], f32, kind="ExternalInput")
    vb_d = nc.dram_tensor("vb", [P, D], f32, kind="ExternalInput")
    gamma_d = nc.dram_tensor("gamma", [P, D], f32, kind="ExternalInput")
    beta_d = nc.dram_tensor("beta", [P, D], f32, kind="ExternalInput")
    y_d = nc.dram_tensor("y", [SL, D], f32, kind="ExternalOutput")

    with (
        tile.TileContext(nc) as tc,
        nc.allow_low_precision(reason="fp8 attention path, tolerance 2e-2"),
        tc.tile_pool(name="persist", bufs=1) as pp,
        tc.tile_pool(name="es_pool", bufs=2) as pes,
        tc.tile_pool(name="bc_pool", bufs=2) as pbc,
        tc.tile_pool(name="y_pool", bufs=3) as pdy,
        tc.tile_pool(name="st_pool", bufs=4) as pst,
        tc.tile_pool(name="ps_sc", bufs=2, space="PSUM") as ps_sc,
        tc.tile_pool(name="ps_out", bufs=4, space="PSUM") as ps_out,
        tc.tile_pool(name="ps_misc", bufs=2, space="PSUM") as ps_misc,
    ):
        qT = pp.tile([P, DC, SL], fp8, tag="qT")
        kT = pp.tile([P, DC, S], fp8, tag="kT")
        v = pp.tile([P, KT_TILES, D], fp8, tag="v")
        outT = pp.tile([P, DC, SL], fp8, tag="outT")
        woT = pp.tile([P, DC, D], fp8, tag="woT")
        xT = pp.tile([P, DC, S], fp8, tag="xT")
        wv = pp.tile([P, DC, D], fp8, tag="wv")
        wqk = [pp.tile([P, DC, P], fp8, name=f"wqk{m}", tag=f"wqk{m}") for m in range(16)]
        if not ZB:
            gamma_bc = pp.tile([P, D], f32, tag="gamma_bc")
            beta_bc = pp.tile([P, D], f32, tag="beta_bc")
            vb_bc = pp.tile([P, D], f32, tag="vb_bc")
            qkvb = pp.tile([P, 24], f32, tag="qkvb")
        ones_k2 = pp.tile([P, 2, P], fp8, tag="ones_k2")
        eps_t = pp.tile([P, 1], f32, tag="eps")
        nln32 = pp.tile([P, 1], f32, tag="nln32")

        nc.vector.memset(ones_k2, 1.0)
        nc.vector.memset(eps_t, EPS)
        nc.vector.memset(nln32, -3.4657359027997265)

        # ---- input DMAs.  All 16 hw DMA engines are SHARED round-robin
        # across everything in flight, so the only way to get the critical
        # bytes early is to not issue the rest yet: wave 1 is exactly what
        # the first six projection chunks touch; everything else is issued
        # from the gpsimd queue behind a data-dependency gate on an early
        # chunk, so it only enters the fabric once wave 1 has landed.
        def xt_piece(eng, dcp, lo, hi):
            eng.dma_start(
                out=xT[:, 2 * dcp : 2 * dcp + 2, lo:hi],
                in_=xT_d[:, 2 * dcp : 2 * dcp + 2, lo:hi],
            )

        for m in (8, 9, 0, 1):
            nc.sync.dma_start(out=wqk[m], in_=wqkT_d[:, m])
        for dcp, eng in ((0, nc.scalar), (1, nc.scalar), (2, nc.gpsimd), (3, nc.gpsimd)):
            xt_piece(eng, dcp, 0, SL)
            xt_piece(eng, dcp, SL, S)
        dma_gate_scr = pp.tile([1, 1], f32, tag="dma_gate_scr")
        if not ZB:
            nc.gpsimd.dma_start(out=qkvb, in_=qkvb_d[:])
            nc.gpsimd.dma_start(out=vb_bc, in_=vb_d[:])
            nc.sync.dma_start(out=gamma_bc, in_=gamma_d[:])
            nc.sync.dma_start(out=beta_bc, in_=beta_d[:])

        # ---- chunk emitters (each: 4 DoubleRow matmuls + one PSUM->SBUF cast) ----
        def qkv_chunk(m, qc, eng, pool=None):
            """Q (m<8, 512 local tokens) or K (m>=8, 512 of 2048 tokens)."""
            pool = pool or ps_misc
            ps = pool.tile([P, 512], f32, name="ps_qk",
                           tag="ps_out" if pool is ps_out else "ps_misc")
            for dcp in range(DC // 2):
                nc.tensor.matmul(
                    ps,
                    lhsT=wqk[m][:, 2 * dcp : 2 * dcp + 2, :],
                    rhs=xT[:, 2 * dcp : 2 * dcp + 2, qc * 512 : (qc + 1) * 512],
                    start=(dcp == 0),
                    stop=(dcp == DC // 2 - 1),
                    perf_mode=DR,
                )
            if m < 8:
                dst = qT[:, m, qc * 512 : (qc + 1) * 512]
            else:
                dst = kT[:, m - 8, qc * 512 : (qc + 1) * 512]
            if ZB:
                if eng is nc.scalar:
                    nc.scalar.activation(
                        out=dst, in_=ps, func=AF.Identity, bias=0.0, scale=1.0
                    )
                else:
                    eng.tensor_copy(out=dst, in_=ps)
            else:
                nc.scalar.activation(
                    out=dst, in_=ps, func=AF.Identity, bias=qkvb[:, m : m + 1], scale=1.0
                )

        def v_chunk(vt, oc, eng, pool=None):
            pool = pool or ps_misc
            ps = pool.tile([P, 512], f32, name="ps_v",
                           tag="ps_out" if pool is ps_out else "ps_misc")
            for dcp in range(DC // 2):
                nc.tensor.matmul(
                    ps,
                    lhsT=xT[:, 2 * dcp : 2 * dcp + 2, vt * P : (vt + 1) * P],
                    rhs=wv[:, 2 * dcp : 2 * dcp + 2, oc * 512 : (oc + 1) * 512],
                    start=(dcp == 0),
                    stop=(dcp == DC // 2 - 1),
                    perf_mode=DR,
                )
            dst = v[:, vt, oc * 512 : (oc + 1) * 512]
            if ZB:
                if eng is nc.scalar:
                    nc.scalar.activation(
                        out=dst, in_=ps, func=AF.Identity, bias=0.0, scale=1.0
                    )
                else:
                    eng.tensor_copy(out=dst, in_=ps)
            else:
                nc.vector.tensor_add(
                    out=dst, in0=ps, in1=vb_bc[:, oc * 512 : (oc + 1) * 512]
                )

        y_tiles = {}

        def wo_chunk(qt, oc, eng, pool=None):
            if qt not in y_tiles:
                y_t = pdy.tile([P, D], f32, name=f"y{qt}", tag="y")
                y_tiles[qt] = y_t
                # residual base: y_t starts as x + wo_b (host-folded); the
                # wo partial sums are added from PSUM by the DVE below, so
                # nothing waits on a DMA in the LayerNorm critical chain.
                nc.gpsimd.dma_start(out=y_t, in_=xb_d[qt * P : (qt + 1) * P, :])
            y_t = y_tiles[qt]
            pool = pool or ps_misc
            ps = pool.tile([P, 512], f32, name="ps_wo",
                           tag="ps_out" if pool is ps_out else "ps_misc")
            for dcp in range(DC // 2):
                nc.tensor.matmul(
                    ps,
                    lhsT=outT[:, 2 * dcp : 2 * dcp + 2, qt * P : (qt + 1) * P],
                    rhs=woT[:, 2 * dcp : 2 * dcp + 2, oc * 512 : (oc + 1) * 512],
                    start=(dcp == 0),
                    stop=(dcp == DC // 2 - 1),
                    perf_mode=DR,
                )
            dst = y_t[:, oc * 512 : (oc + 1) * 512]
            nc.vector.tensor_add(out=dst, in0=ps, in1=dst)

        def ln_tile(qt):
            """LayerNorm + store for token tile qt (residual already in y_t)."""
            y_t = y_tiles.pop(qt)
            stats = pst.tile([P, 2, 6], f32, tag="stats")
            for sg in range(2):
                nc.vector.bn_stats(
                    out=stats[:, sg, :], in_=y_t[:, sg * 512 : (sg + 1) * 512]
                )
            mv = pst.tile([P, 2], f32, tag="mv")
            nc.vector.bn_aggr(out=mv, in_=stats)
            # rstd = exp(-0.5*ln(var+eps)): Ln and Exp share the ACT exp
            # table set, so this never forces a mid-kernel table reload.
            lnv = pst.tile([P, 1], f32, tag="lnv")
            nc.scalar.activation(
                out=lnv, in_=mv[:, 1:2], func=AF.Ln, bias=eps_t, scale=1.0
            )
            rstd = pst.tile([P, 1], f32, tag="rstd")
            nc.scalar.activation(out=rstd, in_=lnv, func=AF.Exp, bias=0.0, scale=-0.5)
            nc.vector.tensor_scalar(
                out=y_t,
                in0=y_t,
                scalar1=mv[:, 0:1],
                scalar2=rstd,
                op0=ALU.subtract,
                op1=ALU.mult,
            )
            if not ZB:
                nc.vector.tensor_mul(out=y_t, in0=y_t, in1=gamma_bc)
                nc.vector.tensor_add(out=y_t, in0=y_t, in1=beta_bc)
            nc.sync.dma_start(out=y_d[qt * P : (qt + 1) * P, :], in_=y_t)

        # ---- softmax finalize (pipelined into the following unit) ----
        def finalize_den(p, on_act=True):
            es_p, ops_p, hqc = p
            dps = ps_misc.tile([P, 512], f32, name="dps", tag="ps_misc")
            for jp in range(KT_TILES // 2):
                nc.tensor.matmul(
                    dps,
                    lhsT=ones_k2,
                    rhs=es_p[:, 2 * jp : 2 * jp + 2, :],
                    start=(jp == 0),
                    stop=(jp == KT_TILES // 2 - 1),
                    perf_mode=DR,
                )
            # 1/den on whichever engine has slack in the surrounding unit:
            # exp(-ln(den)) on ACT (both funcs in the exp table set, no
            # reload) while the DVE is saturated with projection casts, or
            # the DVE reciprocal once ACT becomes the bottleneck.  LUT error
            # ~1e-3, diluted by the residual far below the 2e-2 gate.
            bc = pbc.tile([P, 512], f32, tag="bc")
            if on_act:
                lden = pbc.tile([P, 512], f32, tag="lden")
                nc.scalar.activation(out=lden, in_=dps, func=AF.Ln, bias=0.0, scale=1.0)
                nc.scalar.activation(out=bc, in_=lden, func=AF.Exp, bias=0.0, scale=-1.0)
            else:
                nc.vector.reciprocal(out=bc, in_=dps)
            return (ops_p, hqc, bc)

        def finalize_scale(p2):
            ops_p, (qc_p, h_p), bc = p2
            qsl_p = slice(qc_p * 512, (qc_p + 1) * 512)
            for f in range(2):
                nc.vector.tensor_mul(
                    out=outT[:, 2 * h_p + f, qsl_p], in0=ops_p[f], in1=bc
                )

        # ---- the interleaved main loop ----
        state = {"pend": None, "pend2": None}

        def unit(qc, h, carried, start_at=0, bc_on_act=True):
            """One attention unit (512 q tokens x head h), with `carried`
            thunks (projection/wo/LN work) paced across iterations
            [start_at, 16) of its j-loop."""
            n_car = len(carried)
            emitted = 0
            qsl = slice(qc * 512, (qc + 1) * 512)
            es = pes.tile([P, KT_TILES, 512], fp8, tag="es")
            op0 = ps_out.tile([P, 512], f32, tag="ps_out")
            op1 = ps_out.tile([P, 512], f32, tag="ps_out")
            ops = [op0, op1]
            for j in range(KT_TILES):
                sps = ps_sc.tile([P, 512], f32, tag="ps_sc")
                nc.tensor.matmul(
                    sps,
                    lhsT=kT[:, 2 * h : 2 * h + 2, j * P : (j + 1) * P],
                    rhs=qT[:, 2 * h : 2 * h + 2, qsl],
                    start=True,
                    stop=True,
                    perf_mode=DR,
                )
                nc.scalar.activation(
                    out=es[:, j, :],
                    in_=sps,
                    func=AF.Exp,
                    bias=nln32,
                    scale=1.0 / 16.0,
                )
                if j == 4 and state["pend"] is not None:
                    state["pend2"] = finalize_den(state["pend"], on_act=bc_on_act)
                    state["pend"] = None
                if j == 11 and state["pend2"] is not None:
                    finalize_scale(state["pend2"])
                    state["pend2"] = None
                want = n_car * max(0, j + 1 - start_at) // (KT_TILES - start_at)
                while emitted < want:
                    carried[emitted]()
                    emitted += 1
                if j % 2 == 1:
                    jp = j // 2
                    for f in range(2):
                        nc.tensor.matmul(
                            ops[f],
                            lhsT=v[
                                :,
                                2 * jp : 2 * jp + 2,
                                h * HD + f * P : h * HD + (f + 1) * P,
                            ],
                            rhs=es[:, 2 * jp : 2 * jp + 2, :],
                            start=(jp == 0),
                            stop=(jp == KT_TILES // 2 - 1),
                            perf_mode=DR,
                        )
            while emitted < n_car:
                carried[emitted]()
                emitted += 1
            state["pend"] = (es, ops, (qc, h))

        A, V_ = nc.scalar, nc.vector

        # PRE: everything unit (qc0,h0) needs up front.
        pre = [
            lambda p: qkv_chunk(8, 0, A, p),
            lambda p: qkv_chunk(9, 0, V_, p),
            lambda p: qkv_chunk(0, 0, A, p),
            lambda p: qkv_chunk(1, 0, V_, p),
            lambda p: qkv_chunk(8, 1, A, p),
            lambda p: qkv_chunk(9, 1, V_, p),
            lambda p: qkv_chunk(8, 2, A, p),
            lambda p: qkv_chunk(9, 2, V_, p),
            lambda p: qkv_chunk(8, 3, A, p),
            lambda p: qkv_chunk(9, 3, V_, p),
            lambda p: v_chunk(0, 0, A, p),
            lambda p: v_chunk(1, 0, V_, p),
            lambda p: v_chunk(2, 0, A, p),
            lambda p: v_chunk(3, 0, V_, p),
        ]
        for i, t in enumerate(pre):
            t(ps_out if i % 3 == 2 else ps_misc)
            if i == 1:
                # gate: gpsimd stalls here until this PRE tile exists, so the
                # wave-2 DMAs below don't compete with wave 1 for the fabric.
                nc.gpsimd.tensor_copy(out=dma_gate_scr, in_=kT[0:1, 1, 0:1])
                nc.gpsimd.dma_start(out=wv, in_=wv_d[:])
                for m in (10, 11, 2, 3):
                    nc.gpsimd.dma_start(out=wqk[m], in_=wqkT_d[:, m])

        def carry_k(h2):  # K chunks for head h2 (m = 8+2*h2, 9+2*h2), token order
            out = []
            for qc2 in range(4):
                out.append(lambda m=8 + 2 * h2, q2=qc2: qkv_chunk(m, q2, V_))
                out.append(lambda m=9 + 2 * h2, q2=qc2: qkv_chunk(m, q2, V_))
            return out

        def carry_q(h2, qc2):
            return [
                lambda m=2 * h2: qkv_chunk(m, qc2, V_),
                lambda m=2 * h2 + 1: qkv_chunk(m, qc2, V_),
            ]

        def carry_v(vts, oc):
            return [lambda t=t_, o=oc: v_chunk(t, o, V_) for t_ in vts]

        carried_by_unit = [
            # (qc0,h0): rest of v oc0 (self, paced ahead of av) + head1 k/q
            carry_v(range(4, 16), 0) + carry_k(1) + carry_q(1, 0),
            # (qc0,h1): v oc1 first half + head2 k/q
            carry_v(range(0, 8), 1) + carry_k(2) + carry_q(2, 0),
            # (qc0,h2): v oc1 second half (self-paced, needed from jp>=4) + head3 k/q
            carry_v(range(8, 16), 1) + carry_k(3) + carry_q(3, 0),
            # (qc0,h3): all qc1 q chunks
            [t for h2 in range(4) for t in carry_q(h2, 1)],
            # (qc1,h0): nothing until the qc0 finalize lands (j==11); wo qt0
            [lambda: wo_chunk(0, 0, V_), lambda: wo_chunk(0, 1, V_), lambda: ln_tile(0)],
            # (qc1,h1): wo qt1
            [lambda: wo_chunk(1, 0, V_), lambda: wo_chunk(1, 1, V_), lambda: ln_tile(1)],
            # (qc1,h2): wo qt2
            [lambda: wo_chunk(2, 0, V_), lambda: wo_chunk(2, 1, V_), lambda: ln_tile(2)],
            # (qc1,h3): wo qt3
            [lambda: wo_chunk(3, 0, V_), lambda: wo_chunk(3, 1, V_), lambda: ln_tile(3)],
        ]
        units = [(qc, h) for qc in range(NQ) for h in range(H)]
        for i, ((qc, h), carried) in enumerate(zip(units, carried_by_unit, strict=True)):
            if i == 1:
                for m in (12, 13, 4, 5, 14, 15, 6, 7):
                    nc.gpsimd.dma_start(out=wqk[m], in_=wqkT_d[:, m])
                nc.gpsimd.dma_start(
                    out=woT, in_=woT_d[:].rearrange("(c p) o -> p c o", p=P)
                )
            # (qc1,h0)'s wo work is legal only after the (qc0,h3)
            # finalize_scale, which this unit emits at j==11.
            unit(
                qc,
                h,
                carried,
                start_at=12 if (qc, h) == (1, 0) else 0,
                bc_on_act=(i < 4),
            )

        # drain: finalize last unit, then the remaining 4 token tiles.
        finalize_scale(finalize_den(state["pend"], on_act=True))
        for qt in range(4, 8):
            wo_chunk(qt, 0, A, ps_out)
            wo_chunk(qt, 1, V_, ps_misc)
            ln_tile(qt)

    _split_excess_waits(nc)
    return nc


def _get_nc(ZB):
    key = ("nc", ZB)
    if key not in _CACHE:
        _CACHE[key] = _build(ZB)
    return _CACHE[key]


def kernel(x, qkv_w, qkv_b, wo_w, wo_b, gamma, beta):
    from concourse.bass_utils import run_bass_kernel_spmd

    x = np.asarray(x, dtype=np.float32)
    qkv_w = np.asarray(qkv_w, dtype=np.float32)
    qkv_b = np.asarray(qkv_b, dtype=np.float32)
    wo_w = np.asarray(wo_w, dtype=np.float32)
    wo_b = np.asarray(wo_b, dtype=np.float32)
    gamma = np.asarray(gamma, dtype=np.float32)
    beta = np.asarray(beta, dtype=np.float32)

    W3 = qkv_w.T.reshape(DC, P, 3 * D)  # [c, p, f]
    wqkT = np.ascontiguousarray(
        W3[:, :, : 2 * D].reshape(DC, P, 16, P).transpose(1, 2, 0, 3)
    ).astype(_FP8)  # [p, m, c, j]
    wv_h = np.ascontiguousarray(W3[:, :, 2 * D :].transpose(1, 0, 2)).astype(_FP8)
    woT = np.ascontiguousarray(wo_w.T).astype(_FP8)
    qkvb2d = np.ascontiguousarray(qkv_b.reshape(24, P).T)
    vb = np.ascontiguousarray(np.broadcast_to(qkv_b[2 * D :], (P, D))).astype(np.float32)
    gamma_r = np.ascontiguousarray(np.broadcast_to(gamma, (P, D))).astype(np.float32)
    beta_r = np.ascontiguousarray(np.broadcast_to(beta, (P, D))).astype(np.float32)

    in_maps = []
    for c in range(NCORES):
        b, t = divmod(c, 2)
        loc = x[b, t * SL : (t + 1) * SL]  # [SL, D]
        oth = x[b, (1 - t) * SL : (2 - t) * SL]
        xT = np.concatenate([loc.T, oth.T], axis=1)  # [D, S]
        xT = np.ascontiguousarray(xT.reshape(DC, P, S).transpose(1, 0, 2)).astype(_FP8)
        xb = (loc + wo_b[None, :]).astype(np.float32)
        in_maps.append(
            {
                "xT": xT,
                "wqkT": wqkT,
                "wv": wv_h,
                "woT": woT,
                "xb": xb,
                "qkvb": qkvb2d,
                "vb": vb,
                "gamma": gamma_r,
                "beta": beta_r,
            }
        )

    zb = (
        not qkv_b.any()
        and bool(np.all(gamma == 1.0))
        and not beta.any()
    )
    trace = os.environ.get("BASS_KERNEL_TRACE") == "1"
    res = run_bass_kernel_spmd(
        _get_nc(zb),
        in_maps,
        list(range(NCORES)),
        trace=trace,
        tmpdir=os.environ.get("BASS_KERNEL_TRACE_DIR") or None,
    )
    _CACHE["last_result"] = res

    out = np.empty((B, S, D), dtype=np.float32)
    for c in range(NCORES):
        b, t = divmod(c, 2)
        out[b, t * SL : (t + 1) * SL] = res.results[c]["y"]
    return out
